# revision 1
# baseline (speedup 1.0000x reference)
"""MultiHeadCrossAttention on 8 TRN2 NeuronCores.

Sharding: tensor-parallel over heads (16 heads -> 2 per core).
All activations live transposed ([features, tokens]) on device so every
matmul contracts over the partition dim with zero on-device transposes of
the big activations (V is PE-transposed per 128-col block, which is cheap).
Per core:
  Q.T = (Wq.T slice).T @ x1.T   [128, 4096]
  K.T, V.T from x2.T            [128, 8192]
  per (batch, qcol-chunk, head): S.T = K @ Q.T ; P.T = exp(S.T/8) ;
    outT[d|den] = [V|1]-chunks.T @ P.T  (ones column gives the softmax
    denominator for free) ; attnT = outT[0:64] * recip(outT[64])
  Y.T partial = (Wo.T row-slice).T @ attnT  [1024, 4096]
Host: pre-tiles inputs for contiguous DMA, sums the 8 partials, adds bo,
transposes back. Emission is software-pipelined: KV-projection of batch
b+1 is emitted before attention of batch b; out-projection is fused per
q-column chunk right after its normalize.
"""
import numpy as np
from contextlib import ExitStack

import concourse.bass as bass
import concourse.mybir as mybir
import concourse.tile as tile
from concourse import bacc
from concourse.bass_utils import run_bass_kernel_spmd

N_CORES = 8
B, SQ, SKV, E, DH = 4, 1024, 2048, 1024, 64
Q_ROWS = B * SQ      # 4096
KV_ROWS = B * SKV    # 8192
EC = E // 128        # 8 contraction chunks
QC = Q_ROWS // 512   # 8 q column chunks
KVC_B = SKV // 128   # 16 kv chunks per batch
GB = SQ // 512       # 2 q chunks per batch
F32R = mybir.dt.float32r
F32 = mybir.dt.float32
Exp = mybir.ActivationFunctionType.Exp

_CACHE = {}


def _build(phases=("proj", "attn", "oproj"), n_reps=1):
    nc = bacc.Bacc("TRN2", target_bir_lowering=False, debug=False,
                   num_devices=N_CORES)
    # host-pretiled inputs: each [.., 128, EC, 512] slab is one contiguous DMA
    x1t = nc.dram_tensor("x1t", [QC, 128, EC, 512], F32R,
                         kind="ExternalInput").ap()
    x2t = nc.dram_tensor("x2t", [KV_ROWS // 512, 128, EC, 512], F32R,
                         kind="ExternalInput").ap()
    wqt = nc.dram_tensor("wqt", [128, EC, 128], F32R, kind="ExternalInput").ap()
    wkt = nc.dram_tensor("wkt", [128, EC, 128], F32R, kind="ExternalInput").ap()
    wvt = nc.dram_tensor("wvt", [128, EC, 128], F32R, kind="ExternalInput").ap()
    wot = nc.dram_tensor("wot", [128, E], F32R, kind="ExternalInput").ap()
    bqv = nc.dram_tensor("bq", [128, 1], F32, kind="ExternalInput").ap()
    bkv = nc.dram_tensor("bk", [128, 1], F32, kind="ExternalInput").ap()
    bvv = nc.dram_tensor("bv", [128, 1], F32, kind="ExternalInput").ap()
    idv = nc.dram_tensor("ident", [128, 128], F32R, kind="ExternalInput").ap()
    onv = nc.dram_tensor("ones", [128, 1], F32R, kind="ExternalInput").ap()
    yt = nc.dram_tensor("yt", [E, Q_ROWS], F32, kind="ExternalOutput").ap()
    yt_r = yt.rearrange("(oc p) q -> p oc q", p=128)

    do_proj = "proj" in phases
    do_attn = "attn" in phases and do_proj
    do_oproj = "oproj" in phases and do_attn

    with tile.TileContext(nc) as tc, ExitStack() as ctx:
        const = ctx.enter_context(tc.tile_pool(name="const", bufs=1))
        persist = ctx.enter_context(tc.tile_pool(name="persist", bufs=1))
        xload = ctx.enter_context(tc.tile_pool(name="xload", bufs=7))
        work = ctx.enter_context(tc.tile_pool(name="work", bufs=3))
        ps_pj = ctx.enter_context(tc.tile_pool(name="ps_pj", bufs=2, space="PSUM"))
        ps_s = ctx.enter_context(tc.tile_pool(name="ps_s", bufs=2, space="PSUM"))
        ps_o = ctx.enter_context(tc.tile_pool(name="ps_o", bufs=2, space="PSUM"))

        wq_sb = const.tile([128, EC, 128], F32R, tag="wq")
        wk_sb = const.tile([128, EC, 128], F32R, tag="wk")
        wv_sb = const.tile([128, EC, 128], F32R, tag="wv")
        wo_sb = const.tile([128, E], F32R, tag="wo")
        bq_sb = const.tile([128, 1], F32, tag="bq")
        bk_sb = const.tile([128, 1], F32, tag="bk")
        bv_sb = const.tile([128, 1], F32, tag="bv")
        id_sb = const.tile([128, 128], F32R, tag="id")
        ones_sb = const.tile([128, 1], F32R, tag="ones1")
        nc.sync.dma_start(wq_sb[:], wqt[:])
        nc.sync.dma_start(wk_sb[:], wkt[:])
        nc.sync.dma_start(wv_sb[:], wvt[:])
        nc.sync.dma_start(wo_sb[:], wot[:])
        nc.sync.dma_start(bq_sb[:], bqv[:])
        nc.sync.dma_start(bk_sb[:], bkv[:])
        nc.sync.dma_start(bv_sb[:], bvv[:])
        nc.sync.dma_start(id_sb[:], idv[:])
        nc.sync.dma_start(ones_sb[:], onv[:])

        for rep in range(n_reps):
            qt_sb = persist.tile([128, Q_ROWS], F32R, tag="qt", name=f"qt_{rep}")
            kt_sb = [persist.tile([128, SKV], F32R, tag=f"kt{b}",
                                  name=f"kt{b}_{rep}") for b in range(B)]
            v_sb = [persist.tile([128, KVC_B, 130], F32R, tag=f"v{b}",
                                 name=f"v{b}_{rep}") for b in range(B)]
            at_sb = [persist.tile([128, SQ], F32R, tag=f"at{b}",
                                  name=f"atz{b}_{rep}") for b in range(B)]

            def proj_q(j):
                for u in range(2):
                    xt = xload.tile([128, EC, 256], F32R, tag="x",
                                    name=f"xq{j}_{u}_{rep}")
                    nc.sync.dma_start(xt[:], x1t[j][:, :, u * 256:(u + 1) * 256])
                    if not do_proj:
                        continue
                    q_ps = ps_pj.tile([128, 256], F32, tag="pj",
                                      name=f"qps{j}_{u}_{rep}")
                    for ec in range(EC):
                        nc.tensor.matmul(q_ps[:], wq_sb[:, ec], xt[:, ec],
                                         start=(ec == 0), stop=(ec == EC - 1))
                    c0 = j * 512 + u * 256
                    nc.vector.tensor_scalar_add(qt_sb[:, c0:c0 + 256],
                                                q_ps[:], bq_sb[:])

            def proj_kv(b, half=None):
                rng = range(SKV // 512) if half is None else \
                    range(half * (SKV // 1024), (half + 1) * (SKV // 1024))
                for jj in rng:
                    j = b * (SKV // 512) + jj
                    for u in range(2):
                        xt = xload.tile([128, EC, 256], F32R, tag="x",
                                        name=f"xt{b}_{jj}_{u}_{rep}")
                        nc.sync.dma_start(xt[:],
                                          x2t[j][:, :, u * 256:(u + 1) * 256])
                        if not do_proj:
                            continue
                        k_ps = ps_pj.tile([128, 256], F32, tag="pj",
                                          name=f"kps{b}_{jj}_{u}_{rep}")
                        for ec in range(EC):
                            nc.tensor.matmul(k_ps[:], wk_sb[:, ec], xt[:, ec],
                                             start=(ec == 0), stop=(ec == EC - 1))
                        c0 = jj * 512 + u * 256
                        nc.vector.tensor_scalar_add(
                            kt_sb[b][:, c0:c0 + 256], k_ps[:], bk_sb[:])
                        v_ps = ps_pj.tile([128, 256], F32, tag="pj",
                                          name=f"vps{b}_{jj}_{u}_{rep}")
                        for ec in range(EC):
                            nc.tensor.matmul(v_ps[:], wv_sb[:, ec], xt[:, ec],
                                             start=(ec == 0), stop=(ec == EC - 1))
                        vt_tmp = work.tile([128, 256], F32R, tag="vt", bufs=3,
                                           name=f"vtt{b}_{jj}_{u}_{rep}")
                        nc.vector.tensor_scalar_add(vt_tmp[:], v_ps[:], bv_sb[:])
                        for t in range(2):
                            kc = jj * 4 + u * 2 + t
                            vtp = ps_pj.tile([128, 128], F32R, tag="pj",
                                             name=f"vtp{b}_{kc}_{rep}")
                            nc.tensor.transpose(vtp[:],
                                                vt_tmp[:, t * 128:(t + 1) * 128],
                                                id_sb[:])
                            dst = v_sb[b][:, kc].rearrange("p (h x) -> p h x",
                                                           h=2)
                            nc.vector.tensor_copy(
                                dst[:, :, 0:64],
                                vtp[:].rearrange("p (h x) -> p h x", h=2))

            def oproj_g(b, g):
                if not do_oproj:
                    return
                for o in range(EC):
                    y_ps = ps_pj.tile([128, 512], F32, tag="pj",
                                      name=f"yps{b}_{g}_{o}_{rep}")
                    nc.tensor.matmul(y_ps[:], wo_sb[:, o * 128:(o + 1) * 128],
                                     at_sb[b][:, g * 512:(g + 1) * 512],
                                     start=True, stop=True)
                    y_sb = work.tile([128, 512], F32, tag="y", bufs=3,
                                     name=f"ysb{b}_{g}_{o}_{rep}")
                    nc.vector.tensor_copy(y_sb[:], y_ps[:])
                    nc.sync.dma_start(
                        yt_r[:, o, b * SQ + g * 512: b * SQ + (g + 1) * 512],
                        y_sb[:])

            def attn(b, gsel=None):
                if not do_attn:
                    return
                if gsel in (None, 0):
                    vv = v_sb[b][:].rearrange("p kc (h x) -> p (kc h) x", x=65)
                    nc.vector.tensor_copy(vv[:, :, 64:65],
                                          ones_sb[:].unsqueeze(-1)
                                          .to_broadcast((128, 2 * KVC_B, 1)))
                for g in range(GB) if gsel is None else [gsel]:
                    gs = slice(g * 512, (g + 1) * 512)
                    o_ps = [ps_o.tile([65, 512], F32, tag="o",
                                      name=f"o{b}_{g}_{h}_{rep}")
                            for h in range(2)]
                    for kc in range(0, KVC_B, 2):
                        for h in range(2):
                            hp = h * 64
                            s_ps = ps_s.tile([128, 1024], F32, tag="s",
                                             name=f"sps{b}_{g}_{kc}_{h}_{rep}")
                            pt = work.tile([128, 1024], F32R, tag="pt", bufs=4,
                                           name=f"pt{b}_{g}_{kc}_{h}_{rep}")
                            for u in range(2):
                                nc.tensor.matmul(
                                    s_ps[:, u * 512:(u + 1) * 512],
                                    kt_sb[b][hp:hp + 64,
                                             (kc + u) * 128:(kc + u + 1) * 128],
                                    qt_sb[hp:hp + 64, b * SQ + g * 512:
                                          b * SQ + (g + 1) * 512],
                                    start=True, stop=True)
                            nc.scalar.activation(pt[:], s_ps[:], Exp,
                                                 scale=0.125)
                            for u in range(2):
                                nc.tensor.matmul(
                                    o_ps[h][:],
                                    v_sb[b][:, kc + u, h * 65:h * 65 + 65],
                                    pt[:, u * 512:(u + 1) * 512],
                                    start=(kc == 0 and u == 0),
                                    stop=(kc == KVC_B - 2 and u == 1))
                    for h in range(2):
                        hp = h * 64
                        recip = work.tile([1, 512], F32, tag="recip", bufs=2,
                                          name=f"rc{b}_{g}_{h}_{rep}")
                        nc.vector.reciprocal(recip[:], o_ps[h][64:65, :])
                        rbc = work.tile([64, 512], F32, tag="rbc", bufs=2,
                                        name=f"rbc{b}_{g}_{h}_{rep}")
                        nc.gpsimd.partition_broadcast(rbc[:], recip[:])
                        nc.vector.tensor_mul(at_sb[b][hp:hp + 64, gs],
                                             o_ps[h][0:64, :], rbc[:])
                    oproj_g(b, g)

            # software-pipelined emission: proj(b+1) ahead of attn(b),
            # Q chunks just-in-time (attn(b) needs chunks 2b, 2b+1)
            proj_q(0)
            proj_q(1)
            proj_kv(0)
            for b in range(B):
                if b + 1 < B:
                    proj_q(2 * b + 2)
                    proj_kv(b + 1, half=0)
                    attn(b, gsel=0)
                    proj_q(2 * b + 3)
                    proj_kv(b + 1, half=1)
                    attn(b, gsel=1)
                else:
                    attn(b)

    nc.compile()
    return nc


def _get_nc(phases=("proj", "attn", "oproj"), n_reps=1):
    key = (tuple(phases), n_reps)
    if key not in _CACHE:
        _CACHE[key] = _build(phases, n_reps)
    return _CACHE[key]


def _tile_x(xt2d, nchunks):
    # [E, R] -> [R/512, 128, EC, 512]: x[j, p, ec, q] = xt2d[ec*128+p, j*512+q]
    return np.ascontiguousarray(
        xt2d.reshape(EC, 128, nchunks, 512).transpose(2, 1, 0, 3))


def _tile_w(wt_slice):
    # [E, 128] -> [128, EC, 128]
    return np.ascontiguousarray(
        wt_slice.reshape(EC, 128, 128).transpose(1, 0, 2))


def make_in_maps(x1, x2, Wq, bq, Wk, bk, Wv, bv, Wo, bo=None):
    x1 = np.asarray(x1, dtype=np.float32)
    x2 = np.asarray(x2, dtype=np.float32)
    x1t = _tile_x(np.ascontiguousarray(x1.reshape(Q_ROWS, E).T), QC)
    x2t = _tile_x(np.ascontiguousarray(x2.reshape(KV_ROWS, E).T),
                  KV_ROWS // 512)
    WqT = np.asarray(Wq, dtype=np.float32).T
    WkT = np.asarray(Wk, dtype=np.float32).T
    WvT = np.asarray(Wv, dtype=np.float32).T
    WoT = np.ascontiguousarray(np.asarray(Wo, dtype=np.float32).T)
    ident = np.eye(128, dtype=np.float32)
    ones = np.ones((128, 1), dtype=np.float32)
    in_maps = []
    for c in range(N_CORES):
        s = slice(128 * c, 128 * (c + 1))
        in_maps.append({
            "x1t": x1t, "x2t": x2t,
            "wqt": _tile_w(WqT[:, s]),
            "wkt": _tile_w(WkT[:, s]),
            "wvt": _tile_w(WvT[:, s]),
            "wot": np.ascontiguousarray(WoT[s, :]),
            "bq": np.ascontiguousarray(
                np.asarray(bq, np.float32)[s]).reshape(128, 1),
            "bk": np.ascontiguousarray(
                np.asarray(bk, np.float32)[s]).reshape(128, 1),
            "bv": np.ascontiguousarray(
                np.asarray(bv, np.float32)[s]).reshape(128, 1),
            "ident": ident, "ones": ones,
        })
    return in_maps


def kernel(x1, x2, Wq, bq, Wk, bk, Wv, bv, Wo, bo):
    nc = _get_nc()
    in_maps = make_in_maps(x1, x2, Wq, bq, Wk, bk, Wv, bv, Wo)
    res = run_bass_kernel_spmd(nc, in_maps, list(range(N_CORES)))
    ytf = res.results[0]["yt"].astype(np.float64)
    for c in range(1, N_CORES):
        ytf += res.results[c]["yt"]
    y = ytf.T.astype(np.float32) + np.asarray(bo, np.float32)[None, :]
    return y.reshape(B, SQ, E)



# revision 2
# speedup vs baseline: 1.1219x; 1.1219x over previous
"""MultiHeadCrossAttention on 8 TRN2 NeuronCores.

Sharding: tensor-parallel over heads (16 heads -> 2 per core).
All-bf16 datapath (fp32 PSUM accumulation). Per core, per batch b and
512-wide q group g, a 3-deep software pipeline runs over 8 "k" pieces:
  S(stage s):    S.T[kv,q] = K-slice @ Q.T-slice  (bf16, 2x512 per kc pair)
                 P = exp(S/8) -> bf16 tiles [128kv, 1024]
  PV(stage s-1): flipped full-util matmuls: out[q,65] += P-block.T @ [V|1]
                 (stationary = P [128kv,128q], moving = [V|ones] [128kv,65])
                 normalize with the ones-column denominator, then DMA-XBAR
                 transpose [q,dd] -> [dd,q]
  O(stage s-2):  Y.T[E,q] partial = WoT-slice.T @ attnT, staged to bf16 and
                 DMA'd out; host sums the 8 partials, adds bo, transposes.
V is projected directly in [kv, feat] layout (stationary = X2.T chunks,
moving = WvT) so no on-device V transposes are needed. K/V/Q projections
of batch b+1 are interleaved into the attention pieces of batch b.
"""
import numpy as np
from contextlib import ExitStack

import ml_dtypes

import concourse.bass as bass
import concourse.mybir as mybir
import concourse.tile as tile
from concourse import bacc
from concourse.bass_utils import run_bass_kernel_spmd

N_CORES = 8
B, SQ, SKV, E, DH = 4, 1024, 2048, 1024, 64
Q_ROWS = B * SQ      # 4096
KV_ROWS = B * SKV    # 8192
EC = E // 128        # 8 contraction chunks
QC = Q_ROWS // 512   # 8 q slabs
KVC_B = SKV // 128   # 16 kv chunks (128-wide) per batch
NSLAB = SKV // 512   # 4 kv slabs (512-wide) per batch
BF16 = mybir.dt.bfloat16
F32 = mybir.dt.float32
Exp = mybir.ActivationFunctionType.Exp
NPBF = ml_dtypes.bfloat16

_CACHE = {}


def _build(n_reps=1):
    nc = bacc.Bacc("TRN2", target_bir_lowering=False, debug=False,
                   num_devices=N_CORES)
    x1t = nc.dram_tensor("x1t", [QC, 128, EC, 512], BF16,
                         kind="ExternalInput").ap()
    x2t = nc.dram_tensor("x2t", [B * NSLAB, 128, EC, 512], BF16,
                         kind="ExternalInput").ap()
    wqt = nc.dram_tensor("wqt", [128, EC, 128], BF16, kind="ExternalInput").ap()
    wkt = nc.dram_tensor("wkt", [128, EC, 128], BF16, kind="ExternalInput").ap()
    wvt = nc.dram_tensor("wvt", [128, EC, 128], BF16, kind="ExternalInput").ap()
    wot = nc.dram_tensor("wot", [128, E], BF16, kind="ExternalInput").ap()
    bqv = nc.dram_tensor("bq", [128, 1], F32, kind="ExternalInput").ap()
    bkv = nc.dram_tensor("bk", [128, 1], F32, kind="ExternalInput").ap()
    bvb = nc.dram_tensor("bvb", [128, 128], F32, kind="ExternalInput").ap()
    onv = nc.dram_tensor("ones", [128, 1], F32, kind="ExternalInput").ap()
    yt = nc.dram_tensor("yt", [E, Q_ROWS], BF16, kind="ExternalOutput").ap()
    yt_r = yt.rearrange("(oc p) q -> p oc q", p=128)

    with tile.TileContext(nc) as tc, ExitStack() as ctx:
        const = ctx.enter_context(tc.tile_pool(name="const", bufs=1))
        persist = ctx.enter_context(tc.tile_pool(name="persist", bufs=1))
        xload = ctx.enter_context(tc.tile_pool(name="xload", bufs=5))
        ptp = ctx.enter_context(tc.tile_pool(name="ptp", bufs=32))
        work = ctx.enter_context(tc.tile_pool(name="work", bufs=2))
        ps_s = ctx.enter_context(tc.tile_pool(name="ps_s", bufs=2, space="PSUM"))
        ps_o = ctx.enter_context(tc.tile_pool(name="ps_o", bufs=2, space="PSUM"))
        ps_pj = ctx.enter_context(tc.tile_pool(name="ps_pj", bufs=2,
                                               space="PSUM"))

        wq_sb = const.tile([128, EC, 128], BF16, tag="wq")
        wk_sb = const.tile([128, EC, 128], BF16, tag="wk")
        wv_sb = const.tile([128, EC, 128], BF16, tag="wv")
        wo_sb = const.tile([128, E], BF16, tag="wo")
        bq_sb = const.tile([128, 1], F32, tag="bq")
        bk_sb = const.tile([128, 1], F32, tag="bk")
        bvb_sb = const.tile([128, 128], F32, tag="bvb")
        ones_sb = const.tile([128, 1], F32, tag="ones1")
        nc.sync.dma_start(wq_sb[:], wqt[:])
        nc.sync.dma_start(wk_sb[:], wkt[:])
        nc.sync.dma_start(wv_sb[:], wvt[:])
        nc.sync.dma_start(wo_sb[:], wot[:])
        nc.sync.dma_start(bq_sb[:], bqv[:])
        nc.sync.dma_start(bk_sb[:], bkv[:])
        nc.sync.dma_start(bvb_sb[:], bvb[:])
        nc.sync.dma_start(ones_sb[:], onv[:])

        for rep in range(n_reps):
            qt_sb = persist.tile([128, Q_ROWS], BF16, tag="qt",
                                 name=f"qt_{rep}")
            kt_sb = [persist.tile([128, SKV], BF16, tag=f"kt{b}",
                                  name=f"kt{b}_{rep}") for b in range(B)]
            v_sb = [persist.tile([128, KVC_B, 130], BF16, tag=f"v{b}",
                                 name=f"vz{b}_{rep}") for b in range(B)]
            # ones columns (softmax denominator trick): col 64 of each 65-col
            # [V_h | 1] block
            for b in range(B):
                vv = v_sb[b][:].rearrange("p kc (s y) -> p (kc s) y", y=65)
                nc.vector.tensor_copy(vv[:, :, 64:65],
                                      ones_sb[:].unsqueeze(-1)
                                      .to_broadcast((128, 2 * KVC_B, 1)))

            xstash = {}

            def unit_q(j):
                xt = xload.tile([128, EC, 512], BF16, tag="x",
                                name=f"xq{j}_{rep}")
                nc.sync.dma_start(xt[:], x1t[j])
                ps = ps_pj.tile([128, 512], F32, tag="pj",
                                name=f"qps{j}_{rep}")
                for ec in range(EC):
                    nc.tensor.matmul(ps[:], wq_sb[:, ec], xt[:, ec],
                                     start=(ec == 0), stop=(ec == EC - 1))
                nc.vector.tensor_scalar_add(qt_sb[:, j * 512:(j + 1) * 512],
                                            ps[:], bq_sb[:])

            def unit_k(b, jj):
                xt = xload.tile([128, EC, 512], BF16, tag="x",
                                name=f"xkv{b}_{jj}_{rep}")
                nc.sync.dma_start(xt[:], x2t[b * NSLAB + jj])
                xstash[(b, jj)] = xt
                ps = ps_pj.tile([128, 512], F32, tag="pj",
                                name=f"kps{b}_{jj}_{rep}")
                for ec in range(EC):
                    nc.tensor.matmul(ps[:], wk_sb[:, ec], xt[:, ec],
                                     start=(ec == 0), stop=(ec == EC - 1))
                nc.vector.tensor_scalar_add(
                    kt_sb[b][:, jj * 512:(jj + 1) * 512], ps[:], bk_sb[:])

            def unit_v(b, jj):
                xt = xstash.pop((b, jj))
                ps = ps_pj.tile([128, 512], F32, tag="pj",
                                name=f"vps{b}_{jj}_{rep}")
                for t in range(4):
                    st = xt[:, :, t * 128:(t + 1) * 128]
                    for ec in range(EC):
                        nc.tensor.matmul(ps[:, t * 128:(t + 1) * 128],
                                         st[:, ec], wv_sb[:, ec],
                                         start=(ec == 0), stop=(ec == EC - 1))
                psv = ps[:].rearrange("p (kc s y) -> p kc s y", kc=4, s=2)
                dst = v_sb[b][:, jj * 4:(jj + 1) * 4].rearrange(
                    "p kc (s y) -> p kc s y", s=2)[:, :, :, 0:64]
                bvv = bvb_sb[:].rearrange("p (s y) -> p s y", s=2) \
                    .unsqueeze(1).to_broadcast((128, 4, 2, 64))
                nc.vector.tensor_add(dst, psv, bvv)

            pt_tiles = {}
            atq_tiles = {}
            att_tiles = {}
            ysb_tiles = {}

            def s_piece(si, k):
                b, g = divmod(si, 2)
                qs = slice(si * 512, (si + 1) * 512)
                for h in range(2):
                    hp = h * 64
                    sp = ps_s.tile([128, 1024], F32, tag="s",
                                   name=f"sps{si}_{k}_{h}_{rep}")
                    for u in range(2):
                        kc = 2 * k + u
                        nc.tensor.matmul(
                            sp[:, u * 512:(u + 1) * 512],
                            kt_sb[b][hp:hp + 64, kc * 128:(kc + 1) * 128],
                            qt_sb[hp:hp + 64, qs],
                            start=True, stop=True)
                    pt = ptp.tile([128, 1024], BF16, tag="pt",
                                  name=f"pt{si}_{k}_{h}_{rep}")
                    nc.scalar.activation(pt[:], sp[:], Exp, scale=0.125)
                    pt_tiles[(si, k, h)] = pt

            def pv_group(si, k):
                b, g = divmod(si, 2)
                qb, h = divmod(k, 2)
                op = ps_o.tile([128, 65], F32, tag="o",
                               name=f"ops{si}_{k}_{rep}")
                c0 = qb * 128
                for kp in range(8):
                    pt = pt_tiles[(si, kp, h)]
                    for u in range(2):
                        nc.tensor.matmul(
                            op[:],
                            pt[:, u * 512 + c0:u * 512 + c0 + 128],
                            v_sb[b][:, 2 * kp + u, h * 65:h * 65 + 65],
                            start=(kp == 0 and u == 0),
                            stop=(kp == 7 and u == 1))
                rc = work.tile([128, 1], F32, tag="rc", bufs=4,
                               name=f"rc{si}_{k}_{rep}")
                nc.vector.reciprocal(rc[:], op[:, 64:65])
                if h == 0:
                    atq_tiles[(si, qb)] = work.tile(
                        [128, 128], BF16, tag="atq", bufs=6,
                        name=f"atq{si}_{qb}_{rep}")
                nc.vector.tensor_scalar_mul(
                    atq_tiles[(si, qb)][:, h * 64:(h + 1) * 64],
                    op[:, 0:64], rc[:])

            def transpose_piece(si, qb):
                if qb == 0:
                    att_tiles[si] = work.tile([128, 512], BF16, tag="att",
                                              bufs=3, name=f"att{si}_{rep}")
                nc.sync.dma_start(
                    att_tiles[si][:, qb * 128:(qb + 1) * 128],
                    atq_tiles.pop((si, qb))[:], transpose=True)

            def oproj_piece(si, oc):
                yp = ps_pj.tile([128, 512], F32, tag="pj",
                                name=f"yps{si}_{oc}_{rep}")
                nc.tensor.matmul(yp[:], wo_sb[:, oc * 128:(oc + 1) * 128],
                                 att_tiles[si][:], start=True, stop=True)
                if oc == 0:
                    ysb_tiles[si] = work.tile([128, EC, 512], BF16, tag="ysb",
                                              bufs=2, name=f"ysb{si}_{rep}")
                nc.vector.tensor_copy(ysb_tiles[si][:, oc], yp[:])
                if oc == EC - 1:
                    nc.sync.dma_start(
                        yt_r[:, :, si * 512:(si + 1) * 512],
                        ysb_tiles.pop(si)[:])
                    del att_tiles[si]

            # per-slot projection unit lists (deadline-safe schedule)
            slot_units = [
                [("k", 0, 1), ("v", 0, 1), ("k", 0, 2), ("v", 0, 2),
                 ("k", 0, 3), ("v", 0, 3)],
                [("q", 2, 0), ("k", 1, 0), ("v", 1, 0), ("k", 1, 1),
                 ("v", 1, 1)],
                [("q", 3, 0), ("k", 1, 2), ("v", 1, 2), ("k", 1, 3),
                 ("v", 1, 3)],
                [("q", 4, 0), ("k", 2, 0), ("v", 2, 0), ("k", 2, 1),
                 ("v", 2, 1)],
                [("q", 5, 0), ("k", 2, 2), ("v", 2, 2), ("k", 2, 3),
                 ("v", 2, 3)],
                [("q", 6, 0), ("k", 3, 0), ("v", 3, 0), ("k", 3, 1),
                 ("v", 3, 1)],
                [("q", 7, 0), ("k", 3, 2), ("v", 3, 2), ("k", 3, 3),
                 ("v", 3, 3)],
                [], [], [],
            ]

            def run_unit(u):
                kind, a, bb = u
                if kind == "q":
                    unit_q(a)
                elif kind == "k":
                    unit_k(a, bb)
                else:
                    unit_v(a, bb)

            # prologue
            unit_q(0)
            unit_q(1)
            unit_k(0, 0)
            unit_v(0, 0)

            for s in range(10):
                units = list(slot_units[s])
                for k in range(8):
                    if k < len(units):
                        run_unit(units[k])
                    if s < 8:
                        s_piece(s, k)
                    if 1 <= s <= 8:
                        pv_group(s - 1, k)
                        if k % 2 == 1:
                            transpose_piece(s - 1, k // 2)
                    if s >= 2:
                        oproj_piece(s - 2, k)

    nc.compile()
    return nc


def _get_nc(n_reps=1):
    key = n_reps
    if key not in _CACHE:
        _CACHE[key] = _build(n_reps)
    return _CACHE[key]


def _tile_x(xt2d, nchunks):
    # [E, R] -> [R/512, 128, EC, 512] bf16:
    # x[j, p, ec, q] = xt2d[ec*128+p, j*512+q]
    return np.ascontiguousarray(
        xt2d.reshape(EC, 128, nchunks, 512).transpose(2, 1, 0, 3)).astype(NPBF)


def _tile_w(wt_slice):
    # [E, 128] -> [128, EC, 128]
    return np.ascontiguousarray(
        wt_slice.reshape(EC, 128, 128).transpose(1, 0, 2)).astype(NPBF)


def make_in_maps(x1, x2, Wq, bq, Wk, bk, Wv, bv, Wo, bo=None):
    x1 = np.asarray(x1, dtype=np.float32)
    x2 = np.asarray(x2, dtype=np.float32)
    x1t = _tile_x(np.ascontiguousarray(x1.reshape(Q_ROWS, E).T), QC)
    x2t = _tile_x(np.ascontiguousarray(x2.reshape(KV_ROWS, E).T),
                  KV_ROWS // 512)
    WqT = np.asarray(Wq, dtype=np.float32).T
    WkT = np.asarray(Wk, dtype=np.float32).T
    WvT = np.asarray(Wv, dtype=np.float32).T
    WoT = np.ascontiguousarray(np.asarray(Wo, dtype=np.float32).T)
    ones = np.ones((128, 1), dtype=np.float32)
    bqf = np.asarray(bq, np.float32)
    bkf = np.asarray(bk, np.float32)
    bvf = np.asarray(bv, np.float32)
    in_maps = []
    for c in range(N_CORES):
        s = slice(128 * c, 128 * (c + 1))
        in_maps.append({
            "x1t": x1t, "x2t": x2t,
            "wqt": _tile_w(WqT[:, s]),
            "wkt": _tile_w(WkT[:, s]),
            "wvt": _tile_w(WvT[:, s]),
            "wot": np.ascontiguousarray(WoT[s, :]).astype(NPBF),
            "bq": np.ascontiguousarray(bqf[s]).reshape(128, 1),
            "bk": np.ascontiguousarray(bkf[s]).reshape(128, 1),
            "bvb": np.ascontiguousarray(
                np.broadcast_to(bvf[s][None, :], (128, 128))),
            "ones": ones,
        })
    return in_maps


def kernel(x1, x2, Wq, bq, Wk, bk, Wv, bv, Wo, bo):
    nc = _get_nc()
    in_maps = make_in_maps(x1, x2, Wq, bq, Wk, bk, Wv, bv, Wo)
    res = run_bass_kernel_spmd(nc, in_maps, list(range(N_CORES)))
    ytf = res.results[0]["yt"].astype(np.float64)
    for c in range(1, N_CORES):
        ytf += res.results[c]["yt"].astype(np.float64)
    y = ytf.T.astype(np.float32) + np.asarray(bo, np.float32)[None, :]
    return y.reshape(B, SQ, E)


# revision 8
# speedup vs baseline: 1.2656x; 1.1282x over previous
"""MultiHeadCrossAttention on 8 TRN2 NeuronCores.

Sharding: tensor-parallel over heads (16 heads -> 2 per core).
All-bf16 datapath (fp32 PSUM accumulation). Per core, per batch b and
512-wide q group g, a 3-deep software pipeline runs over 8 "k" pieces:
  S(stage s):    S.T[kv,q] = K-slice @ Q.T-slice  (bf16, 2x512 per kc pair)
                 P = exp(S/8) -> bf16 tiles [128kv, 1024]
  PV(stage s-1): flipped full-util matmuls: out[q,65] += P-block.T @ [V|1]
                 (stationary = P [128kv,128q], moving = [V|ones] [128kv,65])
                 normalize with the ones-column denominator, then DMA-XBAR
                 transpose [q,dd] -> [dd,q]
  O(stage s-2):  Y.T[E,q] partial = WoT-slice.T @ attnT, staged to bf16 and
                 DMA'd out; host sums the 8 partials, adds bo, transposes.
V is projected directly in [kv, feat] layout (stationary = X2.T chunks,
moving = WvT) so no on-device V transposes are needed. K/V/Q projections
of batch b+1 are interleaved into the attention pieces of batch b.
"""
import numpy as np
from contextlib import ExitStack

import ml_dtypes

import concourse.bass as bass
import concourse.mybir as mybir
import concourse.tile as tile
from concourse import bacc
from concourse.bass_utils import run_bass_kernel_spmd

N_CORES = 8
B, SQ, SKV, E, DH = 4, 1024, 2048, 1024, 64
Q_ROWS = B * SQ      # 4096
KV_ROWS = B * SKV    # 8192
EC = E // 128        # 8 contraction chunks
QC = Q_ROWS // 512   # 8 q slabs
KVC_B = SKV // 128   # 16 kv chunks (128-wide) per batch
NSLAB = SKV // 512   # 4 kv slabs (512-wide) per batch
BF16 = mybir.dt.bfloat16
F32 = mybir.dt.float32
Exp = mybir.ActivationFunctionType.Exp
NPBF = ml_dtypes.bfloat16

_CACHE = {}


def _build(n_reps=1):
    nc = bacc.Bacc("TRN2", target_bir_lowering=False, debug=False,
                   num_devices=N_CORES)
    x1t = nc.dram_tensor("x1t", [QC, 128, EC, 512], BF16,
                         kind="ExternalInput").ap()
    x2t = nc.dram_tensor("x2t", [B * NSLAB, 128, EC, 512], BF16,
                         kind="ExternalInput").ap()
    wqt = nc.dram_tensor("wqt", [128, EC, 128], BF16, kind="ExternalInput").ap()
    wkt = nc.dram_tensor("wkt", [128, EC, 128], BF16, kind="ExternalInput").ap()
    wvt = nc.dram_tensor("wvt", [128, EC, 128], BF16, kind="ExternalInput").ap()
    wot = nc.dram_tensor("wot", [128, E], BF16, kind="ExternalInput").ap()
    bqv = nc.dram_tensor("bq", [128, 1], F32, kind="ExternalInput").ap()
    bkv = nc.dram_tensor("bk", [128, 1], F32, kind="ExternalInput").ap()
    bvb = nc.dram_tensor("bvb", [128, 128], F32, kind="ExternalInput").ap()
    onv = nc.dram_tensor("ones", [128, 1], F32, kind="ExternalInput").ap()
    yt = nc.dram_tensor("yt", [E, Q_ROWS], BF16, kind="ExternalOutput").ap()
    yt_r = yt.rearrange("(oc p) q -> p oc q", p=128)

    with tile.TileContext(nc) as tc, ExitStack() as ctx:
        const = ctx.enter_context(tc.tile_pool(name="const", bufs=1))
        persist = ctx.enter_context(tc.tile_pool(name="persist", bufs=1))
        xload = ctx.enter_context(tc.tile_pool(name="xload", bufs=5))
        ptp = ctx.enter_context(tc.tile_pool(name="ptp", bufs=32))
        work = ctx.enter_context(tc.tile_pool(name="work", bufs=2))
        ps_s = ctx.enter_context(tc.tile_pool(name="ps_s", bufs=2, space="PSUM"))
        ps_o = ctx.enter_context(tc.tile_pool(name="ps_o", bufs=2, space="PSUM"))
        ps_pj = ctx.enter_context(tc.tile_pool(name="ps_pj", bufs=2,
                                               space="PSUM"))

        wq_sb = const.tile([128, EC, 128], BF16, tag="wq")
        wk_sb = const.tile([128, EC, 128], BF16, tag="wk")
        wv_sb = const.tile([128, EC, 128], BF16, tag="wv")
        wo_sb = const.tile([128, E], BF16, tag="wo")
        bq_sb = const.tile([128, 1], F32, tag="bq")
        bk_sb = const.tile([128, 1], F32, tag="bk")
        bvb_sb = const.tile([128, 128], F32, tag="bvb")
        ones_sb = const.tile([128, 1], F32, tag="ones1")

        for rep in range(n_reps):
            qt_sb = persist.tile([128, Q_ROWS], BF16, tag="qt",
                                 name=f"qt_{rep}")
            kt_sb = [persist.tile([128, SKV], BF16, tag=f"kt{b}",
                                  name=f"kt{b}_{rep}") for b in range(B)]
            v_sb = [persist.tile([128, KVC_B, 130], BF16, tag=f"v{b}",
                                 name=f"vz{b}_{rep}") for b in range(B)]

            xstash = {}

            def unit_q(j, split=False):
                xt = xload.tile([128, EC, 512], BF16, tag="x",
                                name=f"xq{j}_{rep}")
                if split:
                    nc.sync.dma_start(xt[:, 0:2], x1t[j][:, 0:2])
                    nc.sync.dma_start(xt[:, 2:4], x1t[j][:, 2:4])
                    nc.sync.dma_start(xt[:, 4:8], x1t[j][:, 4:8])
                else:
                    nc.sync.dma_start(xt[:], x1t[j])
                ps = ps_pj.tile([128, 512], F32, tag="pj",
                                name=f"qps{j}_{rep}")
                for ec in range(EC):
                    nc.tensor.matmul(ps[:], wq_sb[:, ec], xt[:, ec],
                                     start=(ec == 0), stop=(ec == EC - 1))
                nc.vector.tensor_scalar_add(qt_sb[:, j * 512:(j + 1) * 512],
                                            ps[:], bq_sb[:])

            def unit_k(b, jj, split=False):
                xt = xload.tile([128, EC, 512], BF16, tag="x",
                                name=f"xkv{b}_{jj}_{rep}")
                if split:
                    nc.sync.dma_start(xt[:, 0:2], x2t[b * NSLAB + jj][:, 0:2])
                    nc.sync.dma_start(xt[:, 2:4], x2t[b * NSLAB + jj][:, 2:4])
                    nc.sync.dma_start(xt[:, 4:8], x2t[b * NSLAB + jj][:, 4:8])
                else:
                    nc.sync.dma_start(xt[:], x2t[b * NSLAB + jj])
                xstash[(b, jj)] = xt
                ps = ps_pj.tile([128, 512], F32, tag="pj",
                                name=f"kps{b}_{jj}_{rep}")
                for ec in range(EC):
                    nc.tensor.matmul(ps[:], wk_sb[:, ec], xt[:, ec],
                                     start=(ec == 0), stop=(ec == EC - 1))
                nc.vector.tensor_scalar_add(
                    kt_sb[b][:, jj * 512:(jj + 1) * 512], ps[:], bk_sb[:])

            def unit_v(b, jj):
                xt = xstash.pop((b, jj))
                ps = ps_pj.tile([128, 512], F32, tag="pj",
                                name=f"vps{b}_{jj}_{rep}")
                for t in range(4):
                    st = xt[:, :, t * 128:(t + 1) * 128]
                    for ec in range(EC):
                        nc.tensor.matmul(ps[:, t * 128:(t + 1) * 128],
                                         st[:, ec], wv_sb[:, ec],
                                         start=(ec == 0), stop=(ec == EC - 1))
                psv = ps[:].rearrange("p (kc s y) -> p kc s y", kc=4, s=2)
                dst = v_sb[b][:, jj * 4:(jj + 1) * 4].rearrange(
                    "p kc (s y) -> p kc s y", s=2)[:, :, :, 0:64]
                bvv = bvb_sb[:].rearrange("p (s y) -> p s y", s=2) \
                    .unsqueeze(1).to_broadcast((128, 4, 2, 64))
                nc.vector.tensor_add(dst, psv, bvv)

            pt_tiles = {}
            atq_tiles = {}
            att_tiles = {}
            ysb_tiles = {}

            def s_piece(si, k):
                b, g = divmod(si, 2)
                qs = slice(si * 512, (si + 1) * 512)
                for h in range(2):
                    hp = h * 64
                    sp = ps_s.tile([128, 1024], F32, tag="s",
                                   name=f"sps{si}_{k}_{h}_{rep}")
                    for u in range(2):
                        kc = 2 * k + u
                        nc.tensor.matmul(
                            sp[:, u * 512:(u + 1) * 512],
                            kt_sb[b][hp:hp + 64, kc * 128:(kc + 1) * 128],
                            qt_sb[hp:hp + 64, qs],
                            start=True, stop=True)
                    pt = ptp.tile([128, 1024], BF16, tag="pt",
                                  name=f"pt{si}_{k}_{h}_{rep}")
                    nc.scalar.activation(pt[:], sp[:], Exp, scale=0.125)
                    pt_tiles[(si, k, h)] = pt

            def pv_group(si, k):
                b, g = divmod(si, 2)
                qb, h = divmod(k, 2)
                op = ps_o.tile([128, 65], F32, tag="o",
                               name=f"ops{si}_{k}_{rep}")
                c0 = qb * 128
                for kp in range(8):
                    pt = pt_tiles[(si, kp, h)]
                    for u in range(2):
                        nc.tensor.matmul(
                            op[:],
                            pt[:, u * 512 + c0:u * 512 + c0 + 128],
                            v_sb[b][:, 2 * kp + u, h * 65:h * 65 + 65],
                            start=(kp == 0 and u == 0),
                            stop=(kp == 7 and u == 1))
                rc = work.tile([128, 1], F32, tag="rc", bufs=4,
                               name=f"rc{si}_{k}_{rep}")
                nc.vector.reciprocal(rc[:], op[:, 64:65])
                if h == 0:
                    atq_tiles[(si, qb)] = work.tile(
                        [128, 128], BF16, tag="atq", bufs=6,
                        name=f"atq{si}_{qb}_{rep}")
                nc.vector.tensor_scalar_mul(
                    atq_tiles[(si, qb)][:, h * 64:(h + 1) * 64],
                    op[:, 0:64], rc[:])

            def transpose_piece(si, qb):
                if qb == 0:
                    att_tiles[si] = work.tile([128, 512], BF16, tag="att",
                                              bufs=4, name=f"att{si}_{rep}")
                nc.sync.dma_start(
                    att_tiles[si][:, qb * 128:(qb + 1) * 128],
                    atq_tiles.pop((si, qb))[:], transpose=True)

            def oproj_piece(si, oc):
                yp = ps_pj.tile([128, 512], F32, tag="pj",
                                name=f"yps{si}_{oc}_{rep}")
                nc.tensor.matmul(yp[:], wo_sb[:, oc * 128:(oc + 1) * 128],
                                 att_tiles[si][:], start=True, stop=True)
                if oc == 0:
                    ysb_tiles[si] = work.tile([128, EC, 512], BF16, tag="ysb",
                                              bufs=2, name=f"ysb{si}_{rep}")
                nc.vector.tensor_copy(ysb_tiles[si][:, oc], yp[:])
                cols = slice(si * 512, (si + 1) * 512)
                if si >= 6:
                    # drain stages: store in halves so the tail DMA overlaps
                    if oc == 3:
                        nc.sync.dma_start(yt_r[:, 0:4, cols],
                                          ysb_tiles[si][:, 0:4])
                    elif oc == EC - 1:
                        nc.sync.dma_start(yt_r[:, 4:8, cols],
                                          ysb_tiles.pop(si)[:, 4:8])
                        del att_tiles[si]
                elif oc == EC - 1:
                    nc.sync.dma_start(yt_r[:, :, cols], ysb_tiles.pop(si)[:])
                    del att_tiles[si]

            # per-slot projection unit lists (deadline-safe schedule)
            slot_units = [
                [("k", 0, 1), ("v", 0, 1), ("k", 0, 2), ("v", 0, 2),
                 ("k", 0, 3), ("v", 0, 3)],
                [("q", 2, 0), ("k", 1, 0), ("v", 1, 0), ("k", 1, 1),
                 ("v", 1, 1)],
                [("q", 3, 0), ("k", 1, 2), ("v", 1, 2), ("k", 1, 3),
                 ("v", 1, 3)],
                [("q", 4, 0), ("k", 2, 0), ("v", 2, 0), ("k", 2, 1),
                 ("v", 2, 1)],
                [("q", 5, 0), ("k", 2, 2), ("v", 2, 2), ("k", 2, 3),
                 ("v", 2, 3)],
                [("q", 6, 0), ("k", 3, 0), ("v", 3, 0), ("k", 3, 1),
                 ("v", 3, 1)],
                [("q", 7, 0), ("k", 3, 2), ("v", 3, 2), ("k", 3, 3),
                 ("v", 3, 3)],
                [], [], [], [],
            ]

            def run_unit(u):
                kind, a, bb = u
                if kind == "q":
                    unit_q(a)
                elif kind == "k":
                    unit_k(a, bb)
                else:
                    unit_v(a, bb)

            # prologue: DMA order minimizes time-to-first-matmul
            nc.sync.dma_start(wq_sb[:], wqt[:])
            nc.sync.dma_start(bq_sb[:], bqv[:])
            unit_q(0, split=True)
            nc.sync.dma_start(wk_sb[:], wkt[:])
            nc.sync.dma_start(bk_sb[:], bkv[:])
            unit_k(0, 0, split=True)
            nc.sync.dma_start(wv_sb[:], wvt[:])
            nc.sync.dma_start(bvb_sb[:], bvb[:])
            nc.sync.dma_start(ones_sb[:], onv[:])
            unit_v(0, 0)
            nc.sync.dma_start(wo_sb[:], wot[:])
            unit_q(1)
            # ones columns (softmax denominator trick): col 64 of each
            # 65-col [V_h | 1] block
            for b in range(B):
                vv = v_sb[b][:].rearrange("p kc (s y) -> p (kc s) y", y=65)
                nc.vector.tensor_copy(vv[:, :, 64:65],
                                      ones_sb[:].unsqueeze(-1)
                                      .to_broadcast((128, 2 * KVC_B, 1)))

            for s in range(11):
                units = list(slot_units[s])
                for k in range(8):
                    if k < len(units):
                        run_unit(units[k])
                    if s < 8:
                        s_piece(s, k)
                    if 1 <= s <= 8:
                        pv_group(s - 1, k)
                        if k % 2 == 1:
                            transpose_piece(s - 1, k // 2)
                    if s >= 3:
                        oproj_piece(s - 3, k)

    nc.compile()
    return nc


def _get_nc(n_reps=1):
    key = n_reps
    if key not in _CACHE:
        _CACHE[key] = _build(n_reps)
    return _CACHE[key]


def _tile_x(xt2d, nchunks):
    # [E, R] -> [R/512, 128, EC, 512] bf16:
    # x[j, p, ec, q] = xt2d[ec*128+p, j*512+q]
    return np.ascontiguousarray(
        xt2d.reshape(EC, 128, nchunks, 512).transpose(2, 1, 0, 3)).astype(NPBF)


def _tile_w(wt_slice):
    # [E, 128] -> [128, EC, 128]
    return np.ascontiguousarray(
        wt_slice.reshape(EC, 128, 128).transpose(1, 0, 2)).astype(NPBF)


def make_in_maps(x1, x2, Wq, bq, Wk, bk, Wv, bv, Wo, bo=None):
    x1 = np.asarray(x1, dtype=np.float32)
    x2 = np.asarray(x2, dtype=np.float32)
    x1t = _tile_x(np.ascontiguousarray(x1.reshape(Q_ROWS, E).T), QC)
    x2t = _tile_x(np.ascontiguousarray(x2.reshape(KV_ROWS, E).T),
                  KV_ROWS // 512)
    WqT = np.asarray(Wq, dtype=np.float32).T
    WkT = np.asarray(Wk, dtype=np.float32).T
    WvT = np.asarray(Wv, dtype=np.float32).T
    WoT = np.ascontiguousarray(np.asarray(Wo, dtype=np.float32).T)
    ones = np.ones((128, 1), dtype=np.float32)
    bqf = np.asarray(bq, np.float32)
    bkf = np.asarray(bk, np.float32)
    bvf = np.asarray(bv, np.float32)
    in_maps = []
    for c in range(N_CORES):
        s = slice(128 * c, 128 * (c + 1))
        in_maps.append({
            "x1t": x1t, "x2t": x2t,
            "wqt": _tile_w(WqT[:, s]),
            "wkt": _tile_w(WkT[:, s]),
            "wvt": _tile_w(WvT[:, s]),
            "wot": np.ascontiguousarray(WoT[s, :]).astype(NPBF),
            "bq": np.ascontiguousarray(bqf[s]).reshape(128, 1),
            "bk": np.ascontiguousarray(bkf[s]).reshape(128, 1),
            "bvb": np.ascontiguousarray(
                np.broadcast_to(bvf[s][None, :], (128, 128))),
            "ones": ones,
        })
    return in_maps


def kernel(x1, x2, Wq, bq, Wk, bk, Wv, bv, Wo, bo):
    nc = _get_nc()
    in_maps = make_in_maps(x1, x2, Wq, bq, Wk, bk, Wv, bv, Wo)
    res = run_bass_kernel_spmd(nc, in_maps, list(range(N_CORES)))
    ytf = res.results[0]["yt"].astype(np.float64)
    for c in range(1, N_CORES):
        ytf += res.results[c]["yt"].astype(np.float64)
    y = ytf.T.astype(np.float32) + np.asarray(bo, np.float32)[None, :]
    return y.reshape(B, SQ, E)


# revision 24
# speedup vs baseline: 1.3217x; 1.0443x over previous
"""MultiHeadCrossAttention on 8 TRN2 NeuronCores.

Sharding: tensor-parallel over heads (16 heads -> 2 per core).
All-bf16 datapath (fp32 PSUM accumulation). Per core, per batch b and
512-wide q group g, a 3-deep software pipeline runs over 8 "k" pieces:
  S(stage s):    S.T[kv,q] = K-slice @ Q.T-slice  (bf16, 2x512 per kc pair)
                 P = exp(S/8) -> bf16 tiles [128kv, 1024]
  PV(stage s-1): flipped full-util matmuls: out[q,65] += P-block.T @ [V|1]
                 (stationary = P [128kv,128q], moving = [V|ones] [128kv,65])
                 normalize with the ones-column denominator, then DMA-XBAR
                 transpose [q,dd] -> [dd,q]
  O(stage s-2):  Y.T[E,q] partial = WoT-slice.T @ attnT, staged to bf16 and
                 DMA'd out; host sums the 8 partials, adds bo, transposes.
V is projected directly in [kv, feat] layout (stationary = X2.T chunks,
moving = WvT) so no on-device V transposes are needed. K/V/Q projections
of batch b+1 are interleaved into the attention pieces of batch b.
"""
import numpy as np
from contextlib import ExitStack

import ml_dtypes

import concourse.bass as bass
import concourse.mybir as mybir
import concourse.tile as tile
from concourse import bacc
from concourse.bass_utils import run_bass_kernel_spmd

N_CORES = 8
B, SQ, SKV, E, DH = 4, 1024, 2048, 1024, 64
Q_ROWS = B * SQ      # 4096
KV_ROWS = B * SKV    # 8192
EC = E // 128        # 8 contraction chunks
QC = Q_ROWS // 512   # 8 q slabs
KVC_B = SKV // 128   # 16 kv chunks (128-wide) per batch
NSLAB = SKV // 512   # 4 kv slabs (512-wide) per batch
BF16 = mybir.dt.bfloat16
F32 = mybir.dt.float32
Exp = mybir.ActivationFunctionType.Exp
NPBF = ml_dtypes.bfloat16

_CACHE = {}


def _build(n_reps=1):
    nc = bacc.Bacc("TRN2", target_bir_lowering=False, debug=False,
                   num_devices=N_CORES)
    x1t = nc.dram_tensor("x1t", [QC, 128, EC, 512], BF16,
                         kind="ExternalInput").ap()
    x2t = nc.dram_tensor("x2t", [B * NSLAB, 128, EC, 512], BF16,
                         kind="ExternalInput").ap()
    wqt = nc.dram_tensor("wqt", [128, EC, 128], BF16, kind="ExternalInput").ap()
    wkt = nc.dram_tensor("wkt", [128, EC, 128], BF16, kind="ExternalInput").ap()
    wvt = nc.dram_tensor("wvt", [128, EC, 128], BF16, kind="ExternalInput").ap()
    wot = nc.dram_tensor("wot", [128, E], BF16, kind="ExternalInput").ap()
    bqv = nc.dram_tensor("bq", [128, 1], F32, kind="ExternalInput").ap()
    bkv = nc.dram_tensor("bk", [128, 1], F32, kind="ExternalInput").ap()
    bvb = nc.dram_tensor("bvb", [128, 128], F32, kind="ExternalInput").ap()
    onv = nc.dram_tensor("ones", [128, 1], F32, kind="ExternalInput").ap()
    idv = nc.dram_tensor("ident", [128, 128], BF16, kind="ExternalInput").ap()
    yt = nc.dram_tensor("yt", [E, Q_ROWS], BF16, kind="ExternalOutput").ap()
    yt_r = yt.rearrange("(oc p) q -> p oc q", p=128)

    with tile.TileContext(nc) as tc, ExitStack() as ctx:
        const = ctx.enter_context(tc.tile_pool(name="const", bufs=1))
        persist = ctx.enter_context(tc.tile_pool(name="persist", bufs=1))
        xload = ctx.enter_context(tc.tile_pool(name="xload", bufs=5))
        ptp = ctx.enter_context(tc.tile_pool(name="ptp", bufs=32))
        work = ctx.enter_context(tc.tile_pool(name="work", bufs=2))
        ps_s = ctx.enter_context(tc.tile_pool(name="ps_s", bufs=2, space="PSUM"))
        ps_o = ctx.enter_context(tc.tile_pool(name="ps_o", bufs=2, space="PSUM"))
        ps_pj = ctx.enter_context(tc.tile_pool(name="ps_pj", bufs=2,
                                               space="PSUM"))

        wq_sb = const.tile([128, EC, 128], BF16, tag="wq")
        wk_sb = const.tile([128, EC, 128], BF16, tag="wk")
        wv_sb = const.tile([128, EC, 128], BF16, tag="wv")
        wo_sb = const.tile([128, E], BF16, tag="wo")
        bq_sb = const.tile([128, 1], F32, tag="bq")
        bk_sb = const.tile([128, 1], F32, tag="bk")
        bvb_sb = const.tile([128, 128], F32, tag="bvb")
        ones_sb = const.tile([128, 1], F32, tag="ones1")
        id_sb = const.tile([128, 128], BF16, tag="id")

        for rep in range(n_reps):
            qt_sb = persist.tile([128, Q_ROWS], BF16, tag="qt",
                                 name=f"qt_{rep}")
            kt_sb = [persist.tile([128, SKV], BF16, tag=f"kt{b}",
                                  name=f"kt{b}_{rep}") for b in range(B)]
            v_sb = [persist.tile([128, KVC_B, 130], BF16, tag=f"v{b}",
                                 name=f"vz{b}_{rep}") for b in range(B)]

            xstash = {}

            def unit_q(j, xt=None):
                if xt is None:
                    xt = xload.tile([128, EC, 512], BF16, tag="x",
                                    name=f"xq{j}_{rep}")
                    nc.sync.dma_start(xt[:], x1t[j])
                ps = ps_pj.tile([128, 512], F32, tag="pj",
                                name=f"qps{j}_{rep}")
                for ec in range(EC):
                    nc.tensor.matmul(ps[:], wq_sb[:, ec], xt[:, ec],
                                     start=(ec == 0), stop=(ec == EC - 1))
                nc.vector.tensor_scalar_add(qt_sb[:, j * 512:(j + 1) * 512],
                                            ps[:], bq_sb[:])

            def unit_k(b, jj, xt=None):
                if xt is None:
                    xt = xload.tile([128, EC, 512], BF16, tag="x",
                                    name=f"xkv{b}_{jj}_{rep}")
                    nc.sync.dma_start(xt[:], x2t[b * NSLAB + jj])
                xstash[(b, jj)] = xt
                ps = ps_pj.tile([128, 512], F32, tag="pj",
                                name=f"kps{b}_{jj}_{rep}")
                for ec in range(EC):
                    nc.tensor.matmul(ps[:], wk_sb[:, ec], xt[:, ec],
                                     start=(ec == 0), stop=(ec == EC - 1))
                nc.vector.tensor_scalar_add(
                    kt_sb[b][:, jj * 512:(jj + 1) * 512], ps[:], bk_sb[:])

            def unit_v(b, jj):
                xt = xstash.pop((b, jj))
                ps = ps_pj.tile([128, 512], F32, tag="pj",
                                name=f"vps{b}_{jj}_{rep}")
                for t in range(4):
                    st = xt[:, :, t * 128:(t + 1) * 128]
                    for ec in range(EC):
                        nc.tensor.matmul(ps[:, t * 128:(t + 1) * 128],
                                         st[:, ec], wv_sb[:, ec],
                                         start=(ec == 0), stop=(ec == EC - 1))
                psv = ps[:].rearrange("p (kc s y) -> p kc s y", kc=4, s=2)
                dst = v_sb[b][:, jj * 4:(jj + 1) * 4].rearrange(
                    "p kc (s y) -> p kc s y", s=2)[:, :, :, 0:64]
                bvv = bvb_sb[:].rearrange("p (s y) -> p s y", s=2) \
                    .unsqueeze(1).to_broadcast((128, 4, 2, 64))
                nc.vector.tensor_add(dst, psv, bvv)

            pt_tiles = {}
            atq_tiles = {}
            att_tiles = {}
            ysb_tiles = {}

            def s_piece(si, k):
                b, g = divmod(si, 2)
                qs = slice(si * 512, (si + 1) * 512)
                for h in range(2):
                    hp = h * 64
                    sp = ps_s.tile([128, 1024], F32, tag="s",
                                   name=f"sps{si}_{k}_{h}_{rep}")
                    for u in range(2):
                        kc = 2 * k + u
                        nc.tensor.matmul(
                            sp[:, u * 512:(u + 1) * 512],
                            kt_sb[b][hp:hp + 64, kc * 128:(kc + 1) * 128],
                            qt_sb[hp:hp + 64, qs],
                            start=True, stop=True)
                    pt = ptp.tile([128, 1024], BF16, tag="pt",
                                  name=f"pt{si}_{k}_{h}_{rep}")
                    nc.scalar.activation(pt[:], sp[:], Exp, scale=0.125)
                    pt_tiles[(si, k, h)] = pt

            def pv_group(si, k):
                b, g = divmod(si, 2)
                qb, h = divmod(k, 2)
                op = ps_o.tile([128, 65], F32, tag="o",
                               name=f"ops{si}_{k}_{rep}")
                c0 = qb * 128
                for kp in range(8):
                    pt = pt_tiles[(si, kp, h)]
                    for u in range(2):
                        nc.tensor.matmul(
                            op[:],
                            pt[:, u * 512 + c0:u * 512 + c0 + 128],
                            v_sb[b][:, 2 * kp + u, h * 65:h * 65 + 65],
                            start=(kp == 0 and u == 0),
                            stop=(kp == 7 and u == 1))
                rc = work.tile([128, 1], F32, tag="rc", bufs=4,
                               name=f"rc{si}_{k}_{rep}")
                nc.vector.reciprocal(rc[:], op[:, 64:65])
                if h == 0:
                    atq_tiles[(si, qb)] = work.tile(
                        [128, 128], BF16, tag="atq", bufs=6,
                        name=f"atq{si}_{qb}_{rep}")
                dst = atq_tiles[(si, qb)][:, h * 64:(h + 1) * 64]
                if si == 7:
                    # drain: Act engine is idle once the last exps retire
                    nc.scalar.mul(dst, op[:, 0:64], rc[:])
                else:
                    nc.vector.tensor_scalar_mul(dst, op[:, 0:64], rc[:])

            def transpose_piece(si, qb):
                if qb == 0:
                    att_tiles[si] = work.tile([128, 512], BF16, tag="att",
                                              bufs=4, name=f"att{si}_{rep}")
                dst = att_tiles[si][:, qb * 128:(qb + 1) * 128]
                src = atq_tiles.pop((si, qb))
                if si >= 6:
                    # drain: PE transpose (short latency); steady state uses
                    # the DMA XBAR (latency hidden by pipeline depth)
                    tp = ps_pj.tile([128, 128], BF16, tag="pj", bufs=2,
                                    name=f"tp{si}_{qb}_{rep}")
                    nc.tensor.transpose(tp[:], src[:], id_sb[:])
                    nc.vector.tensor_copy(dst, tp[:])
                else:
                    nc.sync.dma_start(dst, src[:], transpose=True)

            def oproj_piece(si, oc):
                yp = ps_pj.tile([128, 512], F32, tag="pj",
                                name=f"yps{si}_{oc}_{rep}")
                nc.tensor.matmul(yp[:], wo_sb[:, oc * 128:(oc + 1) * 128],
                                 att_tiles[si][:], start=True, stop=True)
                if oc == 0:
                    ysb_tiles[si] = work.tile([128, EC, 512], BF16, tag="ysb",
                                              bufs=3, name=f"ysb{si}_{rep}")
                nc.vector.tensor_copy(ysb_tiles[si][:, oc], yp[:])
                if oc == EC - 1:
                    nc.sync.dma_start(
                        yt_r[:, :, si * 512:(si + 1) * 512],
                        ysb_tiles.pop(si)[:])
                    del att_tiles[si]

            def oproj_qb(si, qb):
                # drain stages: qb-granular so oproj chases the transposes
                if qb == 0:
                    ysb_tiles[si] = work.tile([128, EC, 512], BF16, tag="ysb",
                                              bufs=3, name=f"ysb{si}_{rep}")
                cq = slice(qb * 128, (qb + 1) * 128)
                for half in range(2):
                    yp = ps_pj.tile([128, 512], F32, tag="pj",
                                    name=f"yqps{si}_{qb}_{half}_{rep}")
                    for j in range(4):
                        oc = half * 4 + j
                        nc.tensor.matmul(
                            yp[:, j * 128:(j + 1) * 128],
                            wo_sb[:, oc * 128:(oc + 1) * 128],
                            att_tiles[si][:, cq], start=True, stop=True)
                    ydst = ysb_tiles[si][:, half * 4:(half + 1) * 4, cq]
                    ysrc = yp[:].rearrange("p (j q) -> p j q", j=4)
                    if si == 7:
                        nc.scalar.copy(ydst, ysrc)
                    else:
                        nc.vector.tensor_copy(ydst, ysrc)
                c0 = si * 512
                if qb == 1:
                    nc.sync.dma_start(yt_r[:, :, c0:c0 + 256],
                                      ysb_tiles[si][:, :, 0:256])
                elif qb == 3:
                    nc.sync.dma_start(yt_r[:, :, c0 + 256:c0 + 512],
                                      ysb_tiles.pop(si)[:, :, 256:512])
                    del att_tiles[si]

            # per-slot projection unit lists (deadline-safe schedule)
            slot_units = [
                [("k", 0, 1), ("v", 0, 1), ("k", 0, 2), ("v", 0, 2),
                 ("k", 0, 3), ("v", 0, 3)],
                [("q", 2, 0), ("k", 1, 0), ("v", 1, 0), ("k", 1, 1),
                 ("v", 1, 1)],
                [("q", 3, 0), ("k", 1, 2), ("v", 1, 2), ("k", 1, 3),
                 ("v", 1, 3)],
                [("q", 4, 0), ("k", 2, 0), ("v", 2, 0), ("k", 2, 1),
                 ("v", 2, 1)],
                [("q", 5, 0), ("k", 2, 2), ("v", 2, 2), ("k", 2, 3),
                 ("v", 2, 3)],
                [("q", 6, 0), ("k", 3, 0), ("v", 3, 0), ("k", 3, 1),
                 ("v", 3, 1)],
                [("q", 7, 0), ("k", 3, 2), ("v", 3, 2), ("k", 3, 3),
                 ("v", 3, 3)],
                [], [], [], [],
            ]

            def run_unit(u):
                kind, a, bb = u
                if kind == "q":
                    unit_q(a)
                elif kind == "k":
                    unit_k(a, bb)
                else:
                    unit_v(a, bb)

            # prologue: DMA order minimizes time-to-first-matmul; x slabs
            # stream in 2-ec chunks paced against the consuming matmuls
            xt_q0 = xload.tile([128, EC, 512], BF16, tag="x",
                               name=f"xq0_{rep}")
            xt_k0 = xload.tile([128, EC, 512], BF16, tag="x",
                               name=f"xkv0_0_{rep}")
            nc.sync.dma_start(wq_sb[:, 0:4], wqt[:, 0:4])
            nc.sync.dma_start(xt_q0[:, 0:2], x1t[0][:, 0:2])
            nc.sync.dma_start(wq_sb[:, 4:8], wqt[:, 4:8])
            nc.sync.dma_start(bq_sb[:], bqv[:])
            nc.sync.dma_start(xt_q0[:, 2:4], x1t[0][:, 2:4])
            nc.sync.dma_start(wk_sb[:], wkt[:])
            nc.sync.dma_start(xt_q0[:, 4:8], x1t[0][:, 4:8])
            nc.sync.dma_start(bk_sb[:], bkv[:])
            nc.sync.dma_start(xt_k0[:, 0:2], x2t[0][:, 0:2])
            unit_q(0, xt=xt_q0)
            nc.sync.dma_start(xt_k0[:, 2:4], x2t[0][:, 2:4])
            nc.sync.dma_start(xt_k0[:, 4:8], x2t[0][:, 4:8])
            nc.sync.dma_start(wv_sb[:], wvt[:])
            unit_k(0, 0, xt=xt_k0)
            nc.sync.dma_start(bvb_sb[:], bvb[:])
            nc.sync.dma_start(ones_sb[:], onv[:])
            unit_v(0, 0)
            nc.sync.dma_start(wo_sb[:], wot[:])
            nc.sync.dma_start(id_sb[:], idv[:])
            unit_q(1)
            # ones columns (softmax denominator trick): col 64 of each
            # 65-col [V_h | 1] block
            for b in range(B):
                vv = v_sb[b][:].rearrange("p kc (s y) -> p (kc s) y", y=65)
                nc.vector.tensor_copy(vv[:, :, 64:65],
                                      ones_sb[:].unsqueeze(-1)
                                      .to_broadcast((128, 2 * KVC_B, 1)))

            for s in range(10):
                units = list(slot_units[s])
                for k in range(8):
                    if k < len(units):
                        run_unit(units[k])
                    if s < 8:
                        s_piece(s, k)
                    if 1 <= s <= 8:
                        pv_group(s - 1, k)
                        if k % 2 == 1:
                            transpose_piece(s - 1, k // 2)
                    if 3 <= s <= 8:
                        oproj_piece(s - 3, k)
                    if s == 7 and k in (3, 5, 7):
                        oproj_qb(6, (k - 3) // 2)
                    elif s == 8 and k == 1:
                        oproj_qb(6, 3)
                    elif s == 8 and k in (3, 5, 7):
                        oproj_qb(7, (k - 3) // 2)
                    elif s == 9 and k == 0:
                        oproj_qb(7, 3)

    nc.compile()
    return nc


def _get_nc(n_reps=1):
    key = n_reps
    if key not in _CACHE:
        _CACHE[key] = _build(n_reps)
    return _CACHE[key]


def _tile_x(xt2d, nchunks):
    # [E, R] -> [R/512, 128, EC, 512] bf16:
    # x[j, p, ec, q] = xt2d[ec*128+p, j*512+q]
    return np.ascontiguousarray(
        xt2d.reshape(EC, 128, nchunks, 512).transpose(2, 1, 0, 3)).astype(NPBF)


def _tile_w(wt_slice):
    # [E, 128] -> [128, EC, 128]
    return np.ascontiguousarray(
        wt_slice.reshape(EC, 128, 128).transpose(1, 0, 2)).astype(NPBF)


def make_in_maps(x1, x2, Wq, bq, Wk, bk, Wv, bv, Wo, bo=None):
    x1 = np.asarray(x1, dtype=np.float32)
    x2 = np.asarray(x2, dtype=np.float32)
    x1t = _tile_x(np.ascontiguousarray(x1.reshape(Q_ROWS, E).T), QC)
    x2t = _tile_x(np.ascontiguousarray(x2.reshape(KV_ROWS, E).T),
                  KV_ROWS // 512)
    WqT = np.asarray(Wq, dtype=np.float32).T
    WkT = np.asarray(Wk, dtype=np.float32).T
    WvT = np.asarray(Wv, dtype=np.float32).T
    WoT = np.ascontiguousarray(np.asarray(Wo, dtype=np.float32).T)
    ones = np.ones((128, 1), dtype=np.float32)
    bqf = np.asarray(bq, np.float32)
    bkf = np.asarray(bk, np.float32)
    bvf = np.asarray(bv, np.float32)
    in_maps = []
    for c in range(N_CORES):
        s = slice(128 * c, 128 * (c + 1))
        in_maps.append({
            "x1t": x1t, "x2t": x2t,
            "wqt": _tile_w(WqT[:, s]),
            "wkt": _tile_w(WkT[:, s]),
            "wvt": _tile_w(WvT[:, s]),
            "wot": np.ascontiguousarray(WoT[s, :]).astype(NPBF),
            "bq": np.ascontiguousarray(bqf[s]).reshape(128, 1),
            "bk": np.ascontiguousarray(bkf[s]).reshape(128, 1),
            "bvb": np.ascontiguousarray(
                np.broadcast_to(bvf[s][None, :], (128, 128))),
            "ones": ones,
            "ident": np.eye(128, dtype=NPBF),
        })
    return in_maps


def kernel(x1, x2, Wq, bq, Wk, bk, Wv, bv, Wo, bo):
    nc = _get_nc()
    in_maps = make_in_maps(x1, x2, Wq, bq, Wk, bk, Wv, bv, Wo)
    res = run_bass_kernel_spmd(nc, in_maps, list(range(N_CORES)))
    ytf = res.results[0]["yt"].astype(np.float64)
    for c in range(1, N_CORES):
        ytf += res.results[c]["yt"].astype(np.float64)
    y = ytf.T.astype(np.float32) + np.asarray(bo, np.float32)[None, :]
    return y.reshape(B, SQ, E)
